# revision 1
# baseline (speedup 1.0000x reference)
"""Two-layer GAT + linear head + log_softmax on 8 Trainium2 NeuronCores.

Strategy (graph/data parallel, per sharding hint):
  - Nodes are sharded 12500/core by id range; edges are owned by the core
    that owns their aggregation row (src = edge[0]).
  - Within each core, nodes are sorted by degree and processed in 98 tiles
    of 128 (partition = node). Each node's edges occupy K̂_t padded slots
    along the free dim; padding points at a dedicated table row whose
    feature block is 0 and whose s_dst entry is -1e30 (=> exp weight 0).
  - h_ext = x @ [W | W@a_src | W@a_dst] is computed shard-locally and
    AllGathered so every core can gather arbitrary dst rows.
  - Per tile: one indirect DMA gathers [128, K̂*(d+2)] rows; softmax over
    edge slots runs per-partition (leaky-relu/exp on ACT with per-partition
    bias, max/recip on DVE); the weighted sum accumulates with K̂ fused
    scalar_tensor_tensor ops; elu; layer-0 output is PE-transposed into an
    h0^T buffer feeding layer 1's matmuls; layer-1 output goes through the
    linear head + log_softmax on chip.

Self-contained: hardcodes N=100000, E=3200000, 8 cores.
"""

import numpy as np

NC_CORES = 8
P = 128
N = 100000
SH = N // NC_CORES          # 12500 nodes per core
SHP = SH + 1                # +1 pad row per shard
T = (SH + P - 1) // P       # 98 tiles
POS = T * P                 # 12544 padded positions
PAD_ROW = SH                # global table row of core 0's pad row
NEG = -1.0e30
D0 = 258                    # 256 features + s_src + s_dst
D1 = 130                    # 128 features + s_src + s_dst
ALPHA = 0.2


def _preprocess(edge):
    src = np.asarray(edge[0], dtype=np.int64)
    dst = np.asarray(edge[1], dtype=np.int64)
    deg = np.bincount(src, minlength=N)

    order = np.empty(N, np.int64)          # position of node within its core
    perm = np.empty((NC_CORES, SH), np.int64)  # perm[c, p] = node id
    deg_by_pos = np.empty((NC_CORES, SH), np.int64)
    for c in range(NC_CORES):
        lo = c * SH
        o = np.argsort(deg[lo:lo + SH], kind="stable")
        order[lo + o] = np.arange(SH)
        perm[c] = lo + o
        deg_by_pos[c] = deg[lo + o]

    core_v = np.arange(N) // SH
    trow = (core_v * SHP + order).astype(np.int64)   # table row of node v

    KHAT = np.zeros(T, np.int64)
    for t in range(T):
        hi = min(t * P + P - 1, SH - 1)
        KHAT[t] = deg_by_pos[:, hi].max()
    KHAT = np.maximum(KHAT, 1)
    offs = np.zeros(T + 1, np.int64)
    offs[1:] = np.cumsum(KHAT)
    S = int(offs[-1])

    gpos = core_v[src] * SH + order[src]       # global sorted position
    sortidx = np.argsort(gpos, kind="stable")
    gp = gpos[sortidx]
    dd = trow[dst[sortidx]].astype(np.int32)

    counts = deg_by_pos.reshape(-1)
    starts = np.zeros(NC_CORES * SH + 1, np.int64)
    starts[1:] = np.cumsum(counts)
    j_in = np.arange(len(gp)) - starts[gp]

    c_e = gp // SH
    p_loc = gp % SH
    t_e = p_loc // P
    prow = p_loc % P
    flat = offs[t_e] * P + prow * KHAT[t_e] + j_in

    eidx = np.full((NC_CORES, S * P), PAD_ROW, np.int32)
    eidx[c_e, flat] = dd

    return dict(KHAT=KHAT.tolist(), offs=offs.tolist(), S=S,
                eidx=eidx, perm=perm)


def _build(KHAT, S, variant="full"):
    import concourse.bacc as bacc
    import concourse.bass as bass
    import concourse.mybir as mybir
    from concourse.tile import TileContext
    from concourse.masks import make_identity

    dt = mybir.dt
    AF = mybir.ActivationFunctionType
    ALU = mybir.AluOpType

    nc = bacc.Bacc()

    xT = nc.declare_dram_parameter("xT", [256, POS], dt.float32, isOutput=False)
    eidx_d = nc.declare_dram_parameter("eidx", [S * P], dt.int32, isOutput=False)
    w0e = nc.declare_dram_parameter("w0e", [256, D0], dt.float32, isOutput=False)
    w1e = nc.declare_dram_parameter("w1e", [256, D1], dt.float32, isOutput=False)
    lw = nc.declare_dram_parameter("lw", [128, 40], dt.float32, isOutput=False)
    lb = nc.declare_dram_parameter("lb", [128, 40], dt.float32, isOutput=False)
    logits = nc.declare_dram_parameter("logits", [POS, 40], dt.float32, isOutput=True)

    if variant == "dbg":
        dbg_sh0 = nc.declare_dram_parameter("dbg_sh0", [SHP, D0], dt.float32, isOutput=True)
        dbg_h0sT = nc.declare_dram_parameter("dbg_h0sT", [256, POS], dt.float32, isOutput=True)
        dbg_sh1 = nc.declare_dram_parameter("dbg_sh1", [SHP, D1], dt.float32, isOutput=True)

    sh0 = nc.dram_tensor("sh0", [SHP, D0], dt.float32)
    t0 = nc.dram_tensor("t0", [NC_CORES * SHP, D0], dt.float32, addr_space="Shared")
    sh1 = nc.dram_tensor("sh1", [SHP, D1], dt.float32)
    t1 = nc.dram_tensor("t1", [NC_CORES * SHP, D1], dt.float32, addr_space="Shared")
    h0sT = nc.dram_tensor("h0sT", [256, POS], dt.float32)

    rg = [list(range(NC_CORES))]

    with TileContext(nc) as tc:
        with (
            tc.tile_pool(name="const", bufs=1) as constp,
            tc.tile_pool(name="gpool", bufs=2) as gpool,
            tc.tile_pool(name="ipool", bufs=4) as ipool,
            tc.tile_pool(name="spool", bufs=4) as spool,
            tc.tile_pool(name="hpool", bufs=4) as hpool,
            tc.tile_pool(name="xpool", bufs=4) as xpool,
            tc.tile_pool(name="psA", bufs=2, space="PSUM") as psA,
            tc.tile_pool(name="psT", bufs=2, space="PSUM") as psT,
        ):
            # ---- resident constants ----
            w0a = constp.tile([128, D0], dt.float32, tag="w0a")
            w0b = constp.tile([128, D0], dt.float32, tag="w0b")
            w1a = constp.tile([128, D1], dt.float32, tag="w1a")
            w1b = constp.tile([128, D1], dt.float32, tag="w1b")
            lwt = constp.tile([128, 40], dt.float32, tag="lwt")
            lbt = constp.tile([128, 40], dt.float32, tag="lbt")
            ident = constp.tile([128, 128], dt.float32, tag="ident")
            ssrc0 = constp.tile([128, T], dt.float32, tag="ssrc0")
            ssrc1 = constp.tile([128, T], dt.float32, tag="ssrc1")
            pad0 = constp.tile([1, D0], dt.float32, tag="pad0")
            pad1 = constp.tile([1, D1], dt.float32, tag="pad1")

            nc.sync.dma_start(out=w0a[:], in_=w0e[0:128, :])
            nc.sync.dma_start(out=w0b[:], in_=w0e[128:256, :])
            nc.sync.dma_start(out=w1a[:], in_=w1e[0:128, :])
            nc.sync.dma_start(out=w1b[:], in_=w1e[128:256, :])
            nc.sync.dma_start(out=lwt[:], in_=lw[:, :])
            nc.sync.dma_start(out=lbt[:], in_=lb[:, :])
            make_identity(nc, ident[:])
            nc.gpsimd.memset(pad0[:], 0.0)
            nc.gpsimd.memset(pad0[:, D0 - 1:D0], NEG)
            nc.gpsimd.memset(pad1[:], 0.0)
            nc.gpsimd.memset(pad1[:, D1 - 1:D1], NEG)
            nc.sync.dma_start(out=sh0[SH:SH + 1, :], in_=pad0[:])
            nc.sync.dma_start(out=sh1[SH:SH + 1, :], in_=pad1[:])

            # ---- dense layer: h_ext = in^T-slices @ W_ext ----
            def dense(src_T, wa, wb, Dg, sh, ssrc_all):
                for t in range(T):
                    cols = slice(t * P, (t + 1) * P)
                    xa = xpool.tile([128, 128], dt.float32, tag="xa")
                    xb = xpool.tile([128, 128], dt.float32, tag="xb")
                    nc.sync.dma_start(out=xa[:], in_=src_T[0:128, cols])
                    nc.sync.dma_start(out=xb[:], in_=src_T[128:256, cols])
                    ps = psA.tile([128, Dg], dt.float32, tag="ps")
                    nc.tensor.matmul(ps[:], lhsT=xa[:], rhs=wa[:], start=True, stop=False)
                    nc.tensor.matmul(ps[:], lhsT=xb[:], rhs=wb[:], start=False, stop=True)
                    h = hpool.tile([128, Dg], dt.float32, tag="hdense")
                    nc.scalar.copy(out=h[:], in_=ps[:])
                    nc.vector.tensor_copy(out=ssrc_all[:, t:t + 1], in_=h[:, Dg - 2:Dg - 1])
                    rows = min(SH - t * P, P)
                    nc.sync.dma_start(out=sh[t * P:t * P + rows, :], in_=h[:rows, :])

            # ---- edge layer ----
            def edge_layer(table, Dg, ssrc_all, emit, offs, dbg_cb=None):
                dh = Dg - 2
                for t in range(T):
                    K = KHAT[t]
                    idx = ipool.tile([128, K], dt.int32, tag="idx")
                    nc.sync.dma_start(
                        out=idx[:],
                        in_=eidx_d[offs[t] * P: offs[t] * P + P * K].rearrange(
                            "(p k) -> p k", p=P),
                    )
                    g = gpool.tile([128, K * Dg], dt.float32, tag="g")
                    # HW indirect DMA consumes ONE offset per partition and
                    # streams out.free_size contiguous elements, so gather
                    # one edge-slot column per call.
                    for j in range(K):
                        nc.gpsimd.indirect_dma_start(
                            out=g[:, j * Dg:(j + 1) * Dg],
                            out_offset=None,
                            in_=table[:],
                            in_offset=bass.IndirectOffsetOnAxis(
                                ap=idx[:, j:j + 1], axis=0),
                        )
                    g3 = g[:].rearrange("p (k d) -> p k d", d=Dg)
                    sd = spool.tile([128, K], dt.float32, tag="sd")
                    nc.scalar.copy(
                        out=sd[:].rearrange("p (k o) -> p k o", o=1),
                        in_=g3[:, :, Dg - 1:Dg],
                    )
                    sc0 = spool.tile([128, K], dt.float32, tag="sc0")
                    nc.scalar.activation(
                        out=sc0[:], in_=sd[:], func=AF.Identity,
                        bias=ssrc_all[:, t:t + 1], scale=1.0,
                    )
                    sc = spool.tile([128, K], dt.float32, tag="sc")
                    nc.vector.scalar_tensor_tensor(
                        out=sc[:], in0=sc0[:], scalar=ALPHA, in1=sc0[:],
                        op0=ALU.mult, op1=ALU.max,
                    )
                    m = spool.tile([128, 1], dt.float32, tag="m")
                    nc.vector.reduce_max(out=m[:], in_=sc[:], axis=mybir.AxisListType.X)
                    negm = spool.tile([128, 1], dt.float32, tag="negm")
                    nc.vector.tensor_scalar_mul(negm[:], m[:], -1.0)
                    e = spool.tile([128, K], dt.float32, tag="e")
                    z = spool.tile([128, 1], dt.float32, tag="z")
                    nc.scalar.activation(
                        out=e[:], in_=sc[:], func=AF.Exp,
                        bias=negm[:, 0:1], scale=1.0, accum_out=z[:, 0:1],
                    )
                    rz = spool.tile([128, 1], dt.float32, tag="rz")
                    nc.vector.reciprocal(rz[:], z[:])
                    acc = hpool.tile([128, dh], dt.float32, tag="acc")
                    nc.vector.tensor_scalar(
                        out=acc[:], in0=g3[:, 0, 0:dh], scalar1=e[:, 0:1],
                        scalar2=None, op0=ALU.mult,
                    )
                    for j in range(1, K):
                        nc.vector.scalar_tensor_tensor(
                            out=acc[:], in0=g3[:, j, 0:dh], scalar=e[:, j:j + 1],
                            in1=acc[:], op0=ALU.mult, op1=ALU.add,
                        )
                    hn = hpool.tile([128, dh], dt.float32, tag="hn")
                    nc.scalar.activation(
                        out=hn[:], in_=acc[:], func=AF.Copy,
                        bias=0.0, scale=rz[:, 0:1],
                    )
                    tneg = hpool.tile([128, dh], dt.float32, tag="tneg")
                    nc.vector.tensor_scalar_min(tneg[:], hn[:], 0.0)
                    expm = hpool.tile([128, dh], dt.float32, tag="expm")
                    nc.scalar.activation(out=expm[:], in_=tneg[:], func=AF.Exp, bias=0.0)
                    ho = hpool.tile([128, dh], dt.float32, tag="ho")
                    nc.vector.scalar_tensor_tensor(
                        out=ho[:], in0=expm[:], scalar=-1.0, in1=hn[:],
                        op0=ALU.add, op1=ALU.max,
                    )
                    if dbg_cb is not None:
                        dbg_cb(t, dict(idx=idx, g=g, sd=sd, sc=sc, e=e, z=z,
                                       acc=acc, hn=hn, ho=ho))
                    emit(t, ho)

            # ---- layer-0 emit: transpose into h0sT ----
            def emit0(t, ho):
                cols = slice(t * P, (t + 1) * P)
                for half in range(2):
                    pt = psT.tile([128, 128], dt.float32, tag="pt")
                    nc.tensor.transpose(
                        pt[:], ho[:, half * 128:(half + 1) * 128], ident[:])
                    ta = xpool.tile([128, 128], dt.float32, tag="ta")
                    nc.scalar.copy(out=ta[:], in_=pt[:])
                    nc.sync.dma_start(
                        out=h0sT[half * 128:(half + 1) * 128, cols], in_=ta[:])

            # ---- layer-1 emit: linear head + log_softmax ----
            def emit1(t, ho):
                pt = psT.tile([128, 128], dt.float32, tag="pt")
                nc.tensor.transpose(pt[:], ho[:, 0:128], ident[:])
                h1T = xpool.tile([128, 128], dt.float32, tag="ta")
                nc.scalar.copy(out=h1T[:], in_=pt[:])
                ps40 = psT.tile([128, 40], dt.float32, tag="ps40")
                nc.tensor.matmul(ps40[:], lhsT=h1T[:], rhs=lwt[:], start=True, stop=True)
                lg = hpool.tile([128, 40], dt.float32, tag="lg")
                nc.vector.tensor_tensor(
                    out=lg[:], in0=ps40[:], in1=lbt[:], op=ALU.add)
                m4 = spool.tile([128, 1], dt.float32, tag="m4")
                nc.vector.reduce_max(out=m4[:], in_=lg[:], axis=mybir.AxisListType.X)
                negm4 = spool.tile([128, 1], dt.float32, tag="negm4")
                nc.vector.tensor_scalar_mul(negm4[:], m4[:], -1.0)
                e4 = hpool.tile([128, 40], dt.float32, tag="e4")
                z4 = spool.tile([128, 1], dt.float32, tag="z4")
                nc.scalar.activation(
                    out=e4[:], in_=lg[:], func=AF.Exp,
                    bias=negm4[:, 0:1], scale=1.0, accum_out=z4[:, 0:1],
                )
                lnz = spool.tile([128, 1], dt.float32, tag="lnz")
                nc.scalar.activation(out=lnz[:], in_=z4[:], func=AF.Ln, bias=0.0)
                lgo = hpool.tile([128, 40], dt.float32, tag="lgo")
                nc.vector.tensor_scalar(
                    out=lgo[:], in0=lg[:], scalar1=negm4[:, 0:1],
                    scalar2=lnz[:, 0:1], op0=ALU.add, op1=ALU.subtract,
                )
                nc.sync.dma_start(out=logits[t * P:(t + 1) * P, :], in_=lgo[:])

            offs = [0]
            for k in KHAT:
                offs.append(offs[-1] + k)

            def final_dummy():
                zt = hpool.tile([128, 40], dt.float32, tag="lgo")
                nc.gpsimd.memset(zt[:], 0.0)
                for t in range(T):
                    nc.sync.dma_start(out=logits[t * P:(t + 1) * P, :], in_=zt[:])

            dense(xT, w0a, w0b, D0, sh0, ssrc0)
            nc.gpsimd.collective_compute(
                "AllGather", mybir.AluOpType.bypass,
                ins=[sh0[:]], outs=[t0[:]], replica_groups=rg,
            )
            dbg_cb = None
            if variant == "dbg2":
                DT = 0
                K0 = KHAT[DT]
                d_outs = {}
                for nm, shape, dty in [
                    ("idx", [128, K0], dt.int32), ("g", [128, K0 * D0], dt.float32),
                    ("sd", [128, K0], dt.float32), ("sc", [128, K0], dt.float32),
                    ("e", [128, K0], dt.float32), ("z", [128, 1], dt.float32),
                    ("acc", [128, 256], dt.float32), ("ho", [128, 256], dt.float32),
                ]:
                    d_outs[nm] = nc.declare_dram_parameter(
                        "dbg2_" + nm, shape, dty, isOutput=True)

                def dbg_cb(t, tiles, _d=d_outs, _DT=DT):
                    if t != _DT:
                        return
                    for nm, tl in tiles.items():
                        if nm in _d:
                            nc.sync.dma_start(out=_d[nm][:, :], in_=tl[:])

            if variant == "v0":
                final_dummy()
            else:
                edge_layer(t0, D0, ssrc0, emit0, offs, dbg_cb=dbg_cb)
                if variant == "v1":
                    final_dummy()
                else:
                    dense(h0sT, w1a, w1b, D1, sh1, ssrc1)
                    nc.gpsimd.collective_compute(
                        "AllGather", mybir.AluOpType.bypass,
                        ins=[sh1[:]], outs=[t1[:]], replica_groups=rg,
                    )
                    if variant == "v2":
                        final_dummy()
                    else:
                        edge_layer(t1, D1, ssrc1, emit1, offs)
                    if variant == "dbg":
                        tc.strict_bb_all_engine_barrier()
                        nc.sync.dma_start(out=dbg_sh0[:, :], in_=sh0[:, :])
                        nc.sync.dma_start(out=dbg_h0sT[:, :], in_=h0sT[:, :])
                        nc.sync.dma_start(out=dbg_sh1[:, :], in_=sh1[:, :])

    nc.finalize()
    return nc


def build_all(inputs):
    """Preprocess + build program + per-core input maps."""
    x = np.ascontiguousarray(np.asarray(inputs["x"], dtype=np.float32))
    edge = np.asarray(inputs["edge"])
    W0 = np.asarray(inputs["W0"], dtype=np.float32)
    a0 = np.asarray(inputs["a0"], dtype=np.float32)
    W1 = np.asarray(inputs["W1"], dtype=np.float32)
    a1 = np.asarray(inputs["a1"], dtype=np.float32)
    lin_w = np.asarray(inputs["lin_w"], dtype=np.float32)
    lin_b = np.asarray(inputs["lin_b"], dtype=np.float32)

    pre = _preprocess(edge)

    w0e = np.concatenate([W0, W0 @ a0[:256], W0 @ a0[256:]], axis=1)  # [256,258]
    w1e = np.concatenate([W1, W1 @ a1[:128], W1 @ a1[128:]], axis=1)  # [256,130]
    lb_rep = np.tile(lin_b[None, :], (128, 1)).astype(np.float32)

    in_maps = []
    for c in range(NC_CORES):
        xTc = np.zeros((256, POS), np.float32)
        xTc[:, :SH] = x[pre["perm"][c]].T
        in_maps.append({
            "xT": xTc,
            "eidx": pre["eidx"][c],
            "w0e": w0e, "w1e": w1e,
            "lw": lin_w, "lb": lb_rep,
        })

    nc = _build(pre["KHAT"], pre["S"])
    return nc, in_maps, pre


def _assemble(results, pre):
    out = np.empty((N, 40), np.float32)
    for c in range(NC_CORES):
        out[pre["perm"][c]] = results[c]["logits"][:SH]
    return out


_CACHE = {}


def _ensure_device(max_tries=8, sleep_s=10.0):
    import time
    import jax

    for i in range(max_tries):
        try:
            a = jax.device_put(np.ones(8, np.float32))
            jax.block_until_ready(a + 1)
            return
        except Exception:  # noqa: BLE001
            if i == max_tries - 1:
                raise
            time.sleep(sleep_s)


def kernel(**inputs) -> np.ndarray:
    import time
    from concourse.bass_utils import run_bass_kernel_spmd

    nc, in_maps, pre = build_all(inputs)
    _ensure_device()
    last = None
    for _ in range(3):
        try:
            res = run_bass_kernel_spmd(nc, in_maps, list(range(NC_CORES)))
            return _assemble(res.results, pre)
        except Exception as e:  # noqa: BLE001
            last = e
            time.sleep(15.0)
            _ensure_device()
    raise last

